# revision 15
# baseline (speedup 1.0000x reference)
"""Trainium2 Bass kernel for nn_NeuralDevice (segment_reduce), v4.

Architecture (per reference):
  two "eyes": h = relu(x @ Wr + br)            [N=1M, 64] -> [N, 128]
              segment-mean over idx (B=65536)  -> [B, 128]
              e = relu(mean @ Wc + bc)         -> [B, 128]
  brain:      z = [e0, e1]; out = relu(z@Wb1+bb1) @ Wb2 + bb2 -> [B, 128]

Distribution: shuffle-by-key, 8 cores x 8192 segments.  Host sorts each
core's nodes by segment, prescales each row by 1/max(cnt,1) (segment SUM
== segment MEAN on device), packs chunk PAIRS into the 128 partitions
(row-tiled K=64 matmul pairs, outputs split across PSUM banks), and
ships the one-hot row->segment selector as fp8 (exact 0/1).

Per 128-row chunk: mm1 pair-packed -> h psum; relu (ACT/DVE 1:1)
-> h bf16 SBUF; mm2: meanT_psum[128 feats, 64 segs] += h^T @ sel_fp8.
8-window groups (512 segs) finish: DVE copy psum->sbuf bf16 = meanT,
eT = relu(Wc^T meanT + bc).  Brain MLP interleaved into eye-1 group
fins.  All cross-engine consumers are emitted 1-3 batches after their
producers (task queue) so the PE FIFO never waits on ACT/DVE latency.
"""

import numpy as np
import ml_dtypes

from concourse import bass, mybir
import concourse.bacc as bacc
import concourse.tile as tile
from concourse.bass_utils import run_bass_kernel_spmd

BF16 = ml_dtypes.bfloat16
FP8 = ml_dtypes.float8_e4m3fn

B_FULL = 65536
N_FULL = 1048576
IN_NF = 64
R_OUT = 128
C_OUT = 128
BRAIN_H = 256
BRAIN_OUT = 128

CORES = 8
SEGS = B_FULL // CORES      # 8192 segments per core
WIN = 64                    # segments per accumulation window
WGRP = 8                    # windows per PSUM group (512 segs)
HB = 8                      # chunks per h-psum batch / relu batch
XCOLS = 4096                # packed-x columns per DMA tile (16 pairs)
SELCH = 128                 # chunks per sel DMA tile
MM2_SKEW = 2                # batches between mm1 and its mm2 consumption

f32 = mybir.dt.float32
bf16 = mybir.dt.bfloat16
fp8e4 = mybir.dt.float8e4
RELU = mybir.ActivationFunctionType.Relu


# ----------------------------------------------------------------- planning

def _plan_eye(idx):
    """Per-eye shared window schedule + per-core sorted node placement."""
    n_win = SEGS // WIN
    owner = idx // SEGS
    per_c = {}
    runs = np.zeros((CORES, n_win), np.int64)
    for c in range(CORES):
        nodes = np.flatnonzero(owner == c)
        srel = idx[nodes] - c * SEGS
        order = np.argsort(srel, kind="stable")
        nodes = nodes[order]
        srel = srel[order]
        per_c[c] = (nodes, srel)
        runs[c] = np.bincount(srel // WIN, minlength=n_win)
    win_sizes = ((runs.max(axis=0) + 127) // 128) * 128
    win_sizes = np.maximum(win_sizes, 128)
    if (int(win_sizes.sum()) // 128) % 2:
        win_sizes[-1] += 128
    return win_sizes.tolist(), per_c


def _eye_sched(win_sizes):
    woc = []
    for w, sz in enumerate(win_sizes):
        woc.extend([w] * (sz // 128))
    first = {}
    last = {}
    for c, w in enumerate(woc):
        first.setdefault(w, c)
        last[w] = c
    return woc, first, last


# ------------------------------------------------------------ program build

_NC_CACHE = {}

# packed bf16 weight layout: [128, 1280]
_WOFF = {"wr0": 0, "wr1": 128, "wc0": 256, "wc1": 384, "wb1lo": 512,
         "wb1hi": 768, "wb2lo": 1024, "wb2hi": 1152}
_WCOLS = 1280
# packed f32 bias layout: [128, 5]
_BOFF = {"bc0": 0, "bc1": 1, "bb1a": 2, "bb1b": 3, "bb2": 4}


def _build_nc(key):
    if key in _NC_CACHE:
        return _NC_CACHE[key]
    (ws0, ws1, has_br) = key
    win_sizes = [list(ws0), list(ws1)]
    scheds = [_eye_sched(win_sizes[0]), _eye_sched(win_sizes[1])]
    nchunks = [len(scheds[0][0]), len(scheds[1][0])]

    nc = bacc.Bacc("TRN2", target_bir_lowering=False, debug=False)

    xp_d = [nc.dram_tensor(f"x{e}p", [128, nchunks[e] * 64], bf16,
                           kind="ExternalInput") for e in range(2)]
    sel_d = [nc.dram_tensor(f"sel{e}", [128, nchunks[e] * WIN], fp8e4,
                            kind="ExternalInput") for e in range(2)]
    wpk_d = nc.dram_tensor("wpk", [128, _WCOLS], bf16, kind="ExternalInput")
    bpk_d = nc.dram_tensor("bpk", [128, 5], f32, kind="ExternalInput")
    if has_br:
        invr_d = [nc.dram_tensor(f"invr{e}", [1, nchunks[e] * 128], f32,
                                 kind="ExternalInput") for e in range(2)]
        br_d = [nc.dram_tensor(f"br{e}", [1, R_OUT], bf16,
                               kind="ExternalInput") for e in range(2)]
    outT_d = nc.dram_tensor("outT", [128, SEGS], f32, kind="ExternalOutput")

    with tile.TileContext(nc) as tc:
        with tc.tile_pool(name="consts", bufs=1) as cp:
            wpk_t = cp.tile([128, _WCOLS], bf16, tag="wpk")
            bpk_t = cp.tile([128, 5], f32, tag="bpk")
            nc.sync.dma_start(out=wpk_t[:], in_=wpk_d[:])
            nc.sync.dma_start(out=bpk_t[:], in_=bpk_d[:])

            def W(name, w=128):
                o = _WOFF[name]
                return wpk_t[:, o:o + w]

            def BIAS(name):
                o = _BOFF[name]
                return bpk_t[:, o:o + 1]

            if has_br:
                br_t = [cp.tile([1, R_OUT], bf16, tag=f"br{e}",
                                name=f"br{e}t") for e in range(2)]
                for e in range(2):
                    nc.sync.dma_start(out=br_t[e][:], in_=br_d[e][:])

            eT_t = [cp.tile([128, SEGS], bf16, tag=f"eT{e}", name=f"eT{e}t")
                    for e in range(2)]

            with (
                tc.tile_pool(name="xch", bufs=4) as xpool,
                tc.tile_pool(name="selp", bufs=4) as selp,
                tc.tile_pool(name="hs", bufs=6) as hpool,
                tc.tile_pool(name="fins", bufs=2) as fs,
                tc.tile_pool(name="invp", bufs=2) as invp,
                tc.tile_pool(name="bs", bufs=3) as bs,
                tc.tile_pool(name="hps", bufs=2, space="PSUM") as hpp,
                tc.tile_pool(name="winp", bufs=2, space="PSUM") as wpp,
                tc.tile_pool(name="wcp", bufs=1, space="PSUM") as wcp,
                tc.tile_pool(name="bph", bufs=1, space="PSUM") as bph,
            ):
                relu_ct = 0
                gi = 0                    # global batch iteration counter
                tasks = []                # (due_gi, fn) queue

                def flush(now):
                    i = 0
                    while i < len(tasks):
                        due, fn = tasks[i]
                        if due <= now:
                            tasks.pop(i)
                            fn()
                            i = 0
                        else:
                            i += 1

                # --------------- brain (split into 3 pipeline tasks)
                def brain_a(t):
                    def fn():
                        r0 = t * 512
                        psh_a = bph.tile([128, 512], f32, tag="bph",
                                         name=f"pha{t}")
                        nc.tensor.matmul(out=psh_a[:], lhsT=W("wb1lo"),
                                         rhs=eT_t[0][:, r0:r0 + 512],
                                         start=True, stop=False)
                        nc.tensor.matmul(out=psh_a[:], lhsT=W("wb1hi"),
                                         rhs=eT_t[1][:, r0:r0 + 512],
                                         start=False, stop=True)
                        hTa = bs.tile([128, 512], bf16, tag="hTa",
                                      name=f"hTa{t}")
                        nc.scalar.activation(out=hTa[:], in_=psh_a[:],
                                             func=RELU, bias=BIAS("bb1a"))
                        fn.hTa = hTa
                    return fn

                def brain_b(t, fa):
                    def fn():
                        r0 = t * 512
                        psh_b = bph.tile([128, 512], f32, tag="bph",
                                         name=f"phb{t}")
                        nc.tensor.matmul(
                            out=psh_b[:],
                            lhsT=wpk_t[:, _WOFF["wb1lo"] + 128:
                                       _WOFF["wb1lo"] + 256],
                            rhs=eT_t[0][:, r0:r0 + 512],
                            start=True, stop=False)
                        nc.tensor.matmul(
                            out=psh_b[:],
                            lhsT=wpk_t[:, _WOFF["wb1hi"] + 128:
                                       _WOFF["wb1hi"] + 256],
                            rhs=eT_t[1][:, r0:r0 + 512],
                            start=False, stop=True)
                        hTb = bs.tile([128, 512], bf16, tag="hTb",
                                      name=f"hTb{t}")
                        nc.scalar.activation(out=hTb[:], in_=psh_b[:],
                                             func=RELU, bias=BIAS("bb1b"))
                        fn.hTb = hTb
                    return fn

                def brain_c(t, fa, fb):
                    def fn():
                        r0 = t * 512
                        psy = bph.tile([128, 512], f32, tag="bph",
                                       name=f"py{t}")
                        nc.tensor.matmul(out=psy[:], lhsT=W("wb2lo"),
                                         rhs=fa.hTa[:], start=True,
                                         stop=False)
                        nc.tensor.matmul(out=psy[:], lhsT=W("wb2hi"),
                                         rhs=fb.hTb[:], start=False,
                                         stop=True)
                        ys = bs.tile([128, 512], f32, tag="ys",
                                     name=f"ys{t}")
                        nc.vector.tensor_scalar_add(ys[:], psy[:],
                                                    BIAS("bb2"))
                        nc.sync.dma_start(out=outT_d[:, r0:r0 + 512],
                                          in_=ys[:])
                    return fn

                # --------------- eye fin: wc matmul + eT relu
                def fin_wc(e, g, meanT):
                    def fn():
                        pse = wcp.tile([128, WGRP * WIN], f32, tag="pse",
                                       name=f"pse{e}_{g}")
                        nc.tensor.matmul(out=pse[:], lhsT=W(f"wc{e}"),
                                         rhs=meanT[:], start=True, stop=True)
                        nc.scalar.activation(
                            out=eT_t[e][:, g * 512:(g + 1) * 512],
                            in_=pse[:], func=RELU, bias=BIAS(f"bc{e}"))
                        if e == 1:
                            fa = brain_a(g)
                            fb = brain_b(g, fa)
                            fc = brain_c(g, fa, fb)
                            tasks.append((gi + 1, fa))
                            tasks.append((gi + 2, fb))
                            tasks.append((gi + 3, fc))
                    return fn

                for e in range(2):
                    woc, wfirst, wlast = scheds[e]
                    nch = nchunks[e]
                    xt = None
                    invt = None
                    selt = None
                    wacc = None
                    pend = []          # batches awaiting mm2 emission

                    def emit_mm2(c0, n, hsb):
                        nonlocal wacc, selt
                        for j in range(n):
                            c = c0 + j
                            w = woc[c]
                            g = w // WGRP
                            if c % SELCH == 0:
                                scnt = min(SELCH, nch - c)
                                selt = selp.tile([128, SELCH * WIN], fp8e4,
                                                 tag="selt",
                                                 name=f"selt{e}_{c}")
                                nc.sync.dma_start(
                                    out=selt[:, :scnt * WIN],
                                    in_=sel_d[e][:, c * WIN:(c + scnt) * WIN])
                            off = (c % SELCH) * WIN
                            if c == wfirst[g * WGRP]:
                                wacc = wpp.tile([128, WGRP * WIN], f32,
                                                tag="wacc", name=f"wa{e}_{g}")
                            ws = (w % WGRP) * WIN
                            slot = (j >> 1) + (j & 1) * (HB // 2)
                            nc.tensor.matmul(
                                out=wacc[:, ws:ws + WIN],
                                lhsT=hsb[:, slot * 128:(slot + 1) * 128],
                                rhs=selt[:, off:off + WIN],
                                start=(c == wfirst[w]),
                                stop=(c == wlast[w]),
                            )
                            if c == wlast[w] and w % WGRP == WGRP - 1:
                                meanT = fs.tile([128, WGRP * WIN], bf16,
                                                tag="meanT", name=f"mt{e}_{g}")
                                nc.vector.tensor_copy(meanT[:], wacc[:])
                                tasks.append((gi + 1, fin_wc(e, g, meanT)))

                    for c0 in range(0, nch, HB):
                        n = min(HB, nch - c0)
                        hps = hpp.tile([128, HB * 128], f32, tag="hps",
                                       name=f"hps{e}_{c0}")
                        hsb = hpool.tile([128, HB * 128], bf16, tag="hsb",
                                         name=f"hsb{e}_{c0}")
                        for t in range(n // 2):
                            pair = c0 // 2 + t
                            if pair % (XCOLS // 128) == 0:
                                pbase = pair * 128
                                pcsz = min(XCOLS, nch * 64 - pbase)
                                xt = xpool.tile([128, XCOLS], bf16, tag="xch",
                                                name=f"xch{e}_{pair}")
                                nc.sync.dma_start(
                                    out=xt[:, :pcsz],
                                    in_=xp_d[e][:, pbase:pbase + pcsz])
                                if has_br:
                                    ibase = pair * 256
                                    icsz = min(2 * XCOLS,
                                               nch * 128 - ibase)
                                    invt = invp.tile([1, 2 * XCOLS], f32,
                                                     tag="invr",
                                                     name=f"invr{e}_{pair}")
                                    nc.sync.dma_start(
                                        out=invt[:, :icsz],
                                        in_=invr_d[e][:, ibase:ibase + icsz])
                            col = (pair % (XCOLS // 128)) * 128
                            for half in range(2):
                                # row-tiled pair: A -> bank0 slot t,
                                # B -> bank1 slot HB//2+t (concurrent row
                                # tiles must write different PSUM banks)
                                slot = t + half * (HB // 2)
                                hs = slice(slot * 128, (slot + 1) * 128)
                                pb = half * 64
                                nc.tensor.matmul(
                                    out=hps[:, hs],
                                    lhsT=xt[pb:pb + 64, col:col + 128],
                                    rhs=W(f"wr{e}")[pb:pb + 64, :],
                                    start=True, stop=not has_br,
                                )
                                if has_br:
                                    ic = (pair % (XCOLS // 128)) * 256 \
                                        + half * 128
                                    nc.tensor.matmul(
                                        out=hps[:, hs],
                                        lhsT=invt[0:1, ic:ic + 128],
                                        rhs=br_t[e][0:1, :],
                                        start=False, stop=True,
                                    )
                        spans = [(0, (n // 2) * 128),
                                 ((HB // 2) * 128, (n // 2) * 128)]
                        for si, (hh, hsz) in enumerate(spans):
                            if (relu_ct + si) % 2 < 1:
                                nc.scalar.activation(
                                    out=hsb[:, hh:hh + hsz],
                                    in_=hps[:, hh:hh + hsz], func=RELU)
                            else:
                                nc.vector.tensor_scalar_max(
                                    hsb[:, hh:hh + hsz],
                                    hps[:, hh:hh + hsz], 0.0)
                        relu_ct += 1
                        pend.append((c0, n, hsb))
                        if len(pend) > MM2_SKEW:
                            emit_mm2(*pend.pop(0))
                        flush(gi)
                        gi += 1
                    while pend:
                        emit_mm2(*pend.pop(0))
                        flush(gi)
                        gi += 1
                while tasks:
                    gi += 1
                    flush(gi)

    nc.compile()
    _NC_CACHE[key] = nc
    return nc


# ------------------------------------------------------------------ driver

def _prepare(inputs):
    x = [np.asarray(inputs["x0"], np.float32),
         np.asarray(inputs["x1"], np.float32)]
    idx = [np.asarray(inputs["idx0"]).astype(np.int64),
           np.asarray(inputs["idx1"]).astype(np.int64)]
    br = [np.asarray(inputs["br0"], np.float32),
          np.asarray(inputs["br1"], np.float32)]
    has_br = bool(np.any(br[0]) or np.any(br[1]))

    plans = [_plan_eye(idx[0]), _plan_eye(idx[1])]
    win_sizes = [plans[0][0], plans[1][0]]
    win_base = [np.cumsum([0] + ws) for ws in win_sizes]
    totals = [int(sum(ws)) for ws in win_sizes]
    n_win = SEGS // WIN

    invc = [
        (1.0 / np.maximum(
            np.bincount(idx[e], minlength=B_FULL), 1)).astype(np.float32)
        for e in range(2)
    ]

    wpk = np.zeros((128, _WCOLS), np.float32)
    for e in range(2):
        wr = np.asarray(inputs[f"Wr{e}"], np.float32)
        wpk[:, _WOFF[f"wr{e}"]:_WOFF[f"wr{e}"] + 128] = \
            np.concatenate([wr, wr], axis=0)
        wpk[:, _WOFF[f"wc{e}"]:_WOFF[f"wc{e}"] + 128] = \
            np.asarray(inputs[f"Wc{e}"], np.float32)
    wb1 = np.asarray(inputs["Wb1"], np.float32)
    wb2 = np.asarray(inputs["Wb2"], np.float32)
    wpk[:, _WOFF["wb1lo"]:_WOFF["wb1lo"] + 256] = wb1[0:128]
    wpk[:, _WOFF["wb1hi"]:_WOFF["wb1hi"] + 256] = wb1[128:256]
    wpk[:, _WOFF["wb2lo"]:_WOFF["wb2lo"] + 128] = wb2[0:128]
    wpk[:, _WOFF["wb2hi"]:_WOFF["wb2hi"] + 128] = wb2[128:256]

    bb1 = np.asarray(inputs["bb1"], np.float32)
    bpk = np.zeros((128, 5), np.float32)
    bpk[:, _BOFF["bc0"]] = np.asarray(inputs["bc0"], np.float32)
    bpk[:, _BOFF["bc1"]] = np.asarray(inputs["bc1"], np.float32)
    bpk[:, _BOFF["bb1a"]] = bb1[0:128]
    bpk[:, _BOFF["bb1b"]] = bb1[128:256]
    bpk[:, _BOFF["bb2"]] = np.asarray(inputs["bb2"], np.float32)

    shared = {"wpk": wpk.astype(BF16), "bpk": bpk}
    if has_br:
        for e in range(2):
            shared[f"br{e}"] = br[e].astype(BF16).reshape(1, -1)

    in_maps = []
    for c in range(CORES):
        m = dict(shared)
        for e in range(2):
            nodes, srel = plans[e][1][c]
            total = totals[e]
            nchunks = total // 128
            wid = srel // WIN
            wstart = np.searchsorted(wid, np.arange(n_win))
            pos = np.empty(len(nodes), np.int64)
            for w in range(n_win):
                lo = wstart[w]
                hi = wstart[w + 1] if w + 1 < n_win else len(nodes)
                pos[lo:hi] = win_base[e][w] + np.arange(hi - lo)
            arr = np.zeros((total, IN_NF), np.float32)
            arr[pos] = x[e][nodes] * invc[e][idx[e][nodes]][:, None]
            a3 = arr.reshape(nchunks, 128, IN_NF).astype(BF16)
            xp = np.concatenate([a3[0::2], a3[1::2]], axis=2)
            m[f"x{e}p"] = np.ascontiguousarray(
                xp.transpose(2, 0, 1).reshape(128, total // 2))
            segv = np.full(total, -1.0, np.float32)
            segv[pos] = (srel % WIN).astype(np.float32)
            sel = (segv[:, None] == np.arange(WIN, dtype=np.float32)[None, :])
            m[f"sel{e}"] = np.ascontiguousarray(
                sel.reshape(nchunks, 128, WIN).transpose(1, 0, 2)
                .reshape(128, nchunks * WIN)).astype(FP8)
            if has_br:
                iv = np.zeros(total, np.float32)
                iv[pos] = invc[e][idx[e][nodes]]
                m[f"invr{e}"] = iv.reshape(1, total)
        in_maps.append(m)
    key = (tuple(win_sizes[0]), tuple(win_sizes[1]), has_br)
    return key, in_maps


def _axon_reset():
    try:
        import ctypes

        lib = ctypes.CDLL("/opt/axon/libaxon_pjrt.so")
        lib.axon_reset.restype = ctypes.c_int
        lib.axon_reset()
    except Exception:
        pass


def _run(inputs, trace=False, trace_kwargs=None):
    key, in_maps = _prepare(inputs)
    nc = _build_nc(key)
    try:
        res = run_bass_kernel_spmd(nc, in_maps, list(range(CORES)),
                                   trace=trace, **(trace_kwargs or {}))
    except Exception as e:
        if "UNRECOVERABLE" not in str(e) and "UNAVAILABLE" not in str(e):
            raise
        _axon_reset()
        res = run_bass_kernel_spmd(nc, in_maps, list(range(CORES)),
                                   trace=trace, **(trace_kwargs or {}))
    out = np.concatenate([res.results[c]["outT"].T for c in range(CORES)],
                         axis=0)
    return out.astype(np.float32), res


def kernel(**inputs):
    return _run(inputs)[0]


# revision 16
# speedup vs baseline: 1.5239x; 1.5239x over previous
"""Trainium2 Bass kernel for nn_NeuralDevice (segment_reduce), v4.

Architecture (per reference):
  two "eyes": h = relu(x @ Wr + br)            [N=1M, 64] -> [N, 128]
              segment-mean over idx (B=65536)  -> [B, 128]
              e = relu(mean @ Wc + bc)         -> [B, 128]
  brain:      z = [e0, e1]; out = relu(z@Wb1+bb1) @ Wb2 + bb2 -> [B, 128]

Distribution: shuffle-by-key, 8 cores x 8192 segments.  Host sorts each
core's nodes by segment, prescales each row by 1/max(cnt,1) (segment SUM
== segment MEAN on device), packs chunk PAIRS into the 128 partitions
(row-tiled K=64 matmul pairs, outputs split across PSUM banks), and
ships the one-hot row->segment selector as fp8 (exact 0/1).

Per 128-row chunk: mm1 pair-packed -> h psum; relu (ACT/DVE 1:1)
-> h bf16 SBUF; mm2: meanT_psum[128 feats, 64 segs] += h^T @ sel_fp8.
8-window groups (512 segs) finish: DVE copy psum->sbuf bf16 = meanT,
eT = relu(Wc^T meanT + bc).  Brain MLP interleaved into eye-1 group
fins.  All cross-engine consumers are emitted 1-3 batches after their
producers (task queue) so the PE FIFO never waits on ACT/DVE latency.
"""

import numpy as np
import ml_dtypes

from concourse import bass, mybir
import concourse.bacc as bacc
import concourse.tile as tile
from concourse.bass_utils import run_bass_kernel_spmd

BF16 = ml_dtypes.bfloat16
FP8 = ml_dtypes.float8_e4m3fn

B_FULL = 65536
N_FULL = 1048576
IN_NF = 64
R_OUT = 128
C_OUT = 128
BRAIN_H = 256
BRAIN_OUT = 128

CORES = 8
SEGS = B_FULL // CORES      # 8192 segments per core
WIN = 64                    # segments per accumulation window
WGRP = 8                    # windows per PSUM group (512 segs)
HB = 8                      # chunks per h-psum batch / relu batch
XCOLS = 4096                # packed-x columns per DMA tile (16 pairs)
SELCH = 128                 # chunks per sel DMA tile
MM2_SKEW = 2                # batches between mm1 and its mm2 consumption

f32 = mybir.dt.float32
bf16 = mybir.dt.bfloat16
fp8e4 = mybir.dt.float8e4
RELU = mybir.ActivationFunctionType.Relu


# ----------------------------------------------------------------- planning

def _plan_eye(idx):
    """Per-eye shared window schedule + per-core sorted node placement."""
    n_win = SEGS // WIN
    owner = idx // SEGS
    per_c = {}
    runs = np.zeros((CORES, n_win), np.int64)
    for c in range(CORES):
        nodes = np.flatnonzero(owner == c)
        srel = idx[nodes] - c * SEGS
        order = np.argsort(srel, kind="stable")
        nodes = nodes[order]
        srel = srel[order]
        per_c[c] = (nodes, srel)
        runs[c] = np.bincount(srel // WIN, minlength=n_win)
    win_sizes = ((runs.max(axis=0) + 127) // 128) * 128
    win_sizes = np.maximum(win_sizes, 128)
    if (int(win_sizes.sum()) // 128) % 2:
        win_sizes[-1] += 128
    return win_sizes.tolist(), per_c


def _eye_sched(win_sizes):
    woc = []
    for w, sz in enumerate(win_sizes):
        woc.extend([w] * (sz // 128))
    first = {}
    last = {}
    for c, w in enumerate(woc):
        first.setdefault(w, c)
        last[w] = c
    return woc, first, last


# ------------------------------------------------------------ program build

_NC_CACHE = {}

# packed bf16 weight layout: [128, 1280]
_WOFF = {"wr0": 0, "wr1": 128, "wc0": 256, "wc1": 384, "wb1lo": 512,
         "wb1hi": 768, "wb2lo": 1024, "wb2hi": 1152}
_WCOLS = 1280
# packed f32 bias layout: [128, 5]
_BOFF = {"bc0": 0, "bc1": 1, "bb1a": 2, "bb1b": 3, "bb2": 4}


def _build_nc(key):
    if key in _NC_CACHE:
        return _NC_CACHE[key]
    (ws0, ws1, has_br) = key
    win_sizes = [list(ws0), list(ws1)]
    scheds = [_eye_sched(win_sizes[0]), _eye_sched(win_sizes[1])]
    nchunks = [len(scheds[0][0]), len(scheds[1][0])]

    nc = bacc.Bacc("TRN2", target_bir_lowering=False, debug=False)

    xp_d = [nc.dram_tensor(f"x{e}p", [128, nchunks[e] * 64], bf16,
                           kind="ExternalInput") for e in range(2)]
    sel_d = [nc.dram_tensor(f"sel{e}", [128, nchunks[e] * WIN], fp8e4,
                            kind="ExternalInput") for e in range(2)]
    wpk_d = nc.dram_tensor("wpk", [128, _WCOLS], bf16, kind="ExternalInput")
    bpk_d = nc.dram_tensor("bpk", [128, 5], f32, kind="ExternalInput")
    if has_br:
        invr_d = [nc.dram_tensor(f"invr{e}", [1, nchunks[e] * 128], f32,
                                 kind="ExternalInput") for e in range(2)]
        br_d = [nc.dram_tensor(f"br{e}", [1, R_OUT], bf16,
                               kind="ExternalInput") for e in range(2)]
    outT_d = nc.dram_tensor("outT", [128, SEGS], f32, kind="ExternalOutput")

    with tile.TileContext(nc) as tc:
        with tc.tile_pool(name="consts", bufs=1) as cp:
            wpk_t = cp.tile([128, _WCOLS], bf16, tag="wpk")
            bpk_t = cp.tile([128, 5], f32, tag="bpk")
            nc.sync.dma_start(out=wpk_t[:], in_=wpk_d[:])
            nc.sync.dma_start(out=bpk_t[:], in_=bpk_d[:])

            def W(name, w=128):
                o = _WOFF[name]
                return wpk_t[:, o:o + w]

            def BIAS(name):
                o = _BOFF[name]
                return bpk_t[:, o:o + 1]

            if has_br:
                br_t = [cp.tile([1, R_OUT], bf16, tag=f"br{e}",
                                name=f"br{e}t") for e in range(2)]
                for e in range(2):
                    nc.sync.dma_start(out=br_t[e][:], in_=br_d[e][:])

            eT_t = [cp.tile([128, SEGS], bf16, tag=f"eT{e}", name=f"eT{e}t")
                    for e in range(2)]

            with (
                tc.tile_pool(name="xch", bufs=4) as xpool,
                tc.tile_pool(name="selp", bufs=4) as selp,
                tc.tile_pool(name="hs", bufs=6) as hpool,
                tc.tile_pool(name="fins", bufs=2) as fs,
                tc.tile_pool(name="invp", bufs=2) as invp,
                tc.tile_pool(name="bs", bufs=3) as bs,
                tc.tile_pool(name="hps", bufs=2, space="PSUM") as hpp,
                tc.tile_pool(name="winp", bufs=2, space="PSUM") as wpp,
                tc.tile_pool(name="wcp", bufs=1, space="PSUM") as wcp,
                tc.tile_pool(name="bph", bufs=1, space="PSUM") as bph,
            ):
                relu_ct = 0
                gi = 0                    # global batch iteration counter
                tasks = []                # (due_gi, fn) queue

                def flush(now):
                    i = 0
                    while i < len(tasks):
                        due, fn = tasks[i]
                        if due <= now:
                            tasks.pop(i)
                            fn()
                            i = 0
                        else:
                            i += 1

                # --------------- brain (split into 3 pipeline tasks)
                def brain_a(t):
                    def fn():
                        r0 = t * 512
                        psh_a = bph.tile([128, 512], f32, tag="bph",
                                         name=f"pha{t}")
                        nc.tensor.matmul(out=psh_a[:], lhsT=W("wb1lo"),
                                         rhs=eT_t[0][:, r0:r0 + 512],
                                         start=True, stop=False)
                        nc.tensor.matmul(out=psh_a[:], lhsT=W("wb1hi"),
                                         rhs=eT_t[1][:, r0:r0 + 512],
                                         start=False, stop=True)
                        hTa = bs.tile([128, 512], bf16, tag="hTa",
                                      name=f"hTa{t}")
                        nc.scalar.activation(out=hTa[:], in_=psh_a[:],
                                             func=RELU, bias=BIAS("bb1a"))
                        fn.hTa = hTa
                    return fn

                def brain_b(t, fa):
                    def fn():
                        r0 = t * 512
                        psh_b = bph.tile([128, 512], f32, tag="bph",
                                         name=f"phb{t}")
                        nc.tensor.matmul(
                            out=psh_b[:],
                            lhsT=wpk_t[:, _WOFF["wb1lo"] + 128:
                                       _WOFF["wb1lo"] + 256],
                            rhs=eT_t[0][:, r0:r0 + 512],
                            start=True, stop=False)
                        nc.tensor.matmul(
                            out=psh_b[:],
                            lhsT=wpk_t[:, _WOFF["wb1hi"] + 128:
                                       _WOFF["wb1hi"] + 256],
                            rhs=eT_t[1][:, r0:r0 + 512],
                            start=False, stop=True)
                        hTb = bs.tile([128, 512], bf16, tag="hTb",
                                      name=f"hTb{t}")
                        nc.scalar.activation(out=hTb[:], in_=psh_b[:],
                                             func=RELU, bias=BIAS("bb1b"))
                        fn.hTb = hTb
                    return fn

                def brain_c(t, fa, fb):
                    def fn():
                        r0 = t * 512
                        psy = bph.tile([128, 512], f32, tag="bph",
                                       name=f"py{t}")
                        nc.tensor.matmul(out=psy[:], lhsT=W("wb2lo"),
                                         rhs=fa.hTa[:], start=True,
                                         stop=False)
                        nc.tensor.matmul(out=psy[:], lhsT=W("wb2hi"),
                                         rhs=fb.hTb[:], start=False,
                                         stop=True)
                        ys = bs.tile([128, 512], f32, tag="ys",
                                     name=f"ys{t}")
                        nc.vector.tensor_scalar_add(ys[:], psy[:],
                                                    BIAS("bb2"))
                        nc.sync.dma_start(out=outT_d[:, r0:r0 + 512],
                                          in_=ys[:])
                    return fn

                # --------------- eye fin: wc matmul + eT relu
                def fin_wc(e, g, meanT):
                    def fn():
                        pse = wcp.tile([128, WGRP * WIN], f32, tag="pse",
                                       name=f"pse{e}_{g}")
                        nc.tensor.matmul(out=pse[:], lhsT=W(f"wc{e}"),
                                         rhs=meanT[:], start=True, stop=True)
                        nc.scalar.activation(
                            out=eT_t[e][:, g * 512:(g + 1) * 512],
                            in_=pse[:], func=RELU, bias=BIAS(f"bc{e}"))
                        if e == 1:
                            fa = brain_a(g)
                            fb = brain_b(g, fa)
                            fc = brain_c(g, fa, fb)
                            tasks.append((gi + 1, fa))
                            tasks.append((gi + 2, fb))
                            tasks.append((gi + 3, fc))
                    return fn

                for e in range(2):
                    woc, wfirst, wlast = scheds[e]
                    nch = nchunks[e]
                    xt = None
                    invt = None
                    selt = None
                    wacc = None
                    pend = []          # batches awaiting mm2 emission

                    def emit_mm2(c0, n, hsb):
                        nonlocal wacc, selt
                        for j in range(n):
                            c = c0 + j
                            w = woc[c]
                            g = w // WGRP
                            if c % SELCH == 0:
                                scnt = min(SELCH, nch - c)
                                selt = selp.tile([128, SELCH * WIN], fp8e4,
                                                 tag="selt",
                                                 name=f"selt{e}_{c}")
                                nc.sync.dma_start(
                                    out=selt[:, :scnt * WIN],
                                    in_=sel_d[e][:, c * WIN:(c + scnt) * WIN])
                            off = (c % SELCH) * WIN
                            if c == wfirst[g * WGRP]:
                                wacc = wpp.tile([128, WGRP * WIN], f32,
                                                tag="wacc", name=f"wa{e}_{g}")
                            ws = (w % WGRP) * WIN
                            slot = (j >> 1) + (j & 1) * (HB // 2)
                            nc.tensor.matmul(
                                out=wacc[:, ws:ws + WIN],
                                lhsT=hsb[:, slot * 128:(slot + 1) * 128],
                                rhs=selt[:, off:off + WIN],
                                start=(c == wfirst[w]),
                                stop=(c == wlast[w]),
                            )
                            if c == wlast[w] and w % WGRP == WGRP - 1:
                                meanT = fs.tile([128, WGRP * WIN], bf16,
                                                tag="meanT", name=f"mt{e}_{g}")
                                nc.vector.tensor_copy(meanT[:], wacc[:])
                                tasks.append((gi + 1, fin_wc(e, g, meanT)))

                    for c0 in range(0, nch, HB):
                        n = min(HB, nch - c0)
                        hps = hpp.tile([128, HB * 128], f32, tag="hps",
                                       name=f"hps{e}_{c0}")
                        hsb = hpool.tile([128, HB * 128], bf16, tag="hsb",
                                         name=f"hsb{e}_{c0}")
                        for t in range(n // 2):
                            pair = c0 // 2 + t
                            if pair % (XCOLS // 128) == 0:
                                pbase = pair * 128
                                pcsz = min(XCOLS, nch * 64 - pbase)
                                xt = xpool.tile([128, XCOLS], bf16, tag="xch",
                                                name=f"xch{e}_{pair}")
                                nc.sync.dma_start(
                                    out=xt[:, :pcsz],
                                    in_=xp_d[e][:, pbase:pbase + pcsz])
                                if has_br:
                                    ibase = pair * 256
                                    icsz = min(2 * XCOLS,
                                               nch * 128 - ibase)
                                    invt = invp.tile([1, 2 * XCOLS], f32,
                                                     tag="invr",
                                                     name=f"invr{e}_{pair}")
                                    nc.sync.dma_start(
                                        out=invt[:, :icsz],
                                        in_=invr_d[e][:, ibase:ibase + icsz])
                            col = (pair % (XCOLS // 128)) * 128
                            for half in range(2):
                                # row-tiled pair: A -> bank0 slot t,
                                # B -> bank1 slot HB//2+t (concurrent row
                                # tiles must write different PSUM banks)
                                slot = t + half * (HB // 2)
                                hs = slice(slot * 128, (slot + 1) * 128)
                                pb = half * 64
                                nc.tensor.matmul(
                                    out=hps[:, hs],
                                    lhsT=xt[pb:pb + 64, col:col + 128],
                                    rhs=W(f"wr{e}")[pb:pb + 64, :],
                                    start=True, stop=not has_br,
                                )
                                if has_br:
                                    ic = (pair % (XCOLS // 128)) * 256 \
                                        + half * 128
                                    nc.tensor.matmul(
                                        out=hps[:, hs],
                                        lhsT=invt[0:1, ic:ic + 128],
                                        rhs=br_t[e][0:1, :],
                                        start=False, stop=True,
                                    )
                        if n == HB:
                            spans = [(0, HB * 128)]
                        else:
                            spans = [(0, (n // 2) * 128),
                                     ((HB // 2) * 128, (n // 2) * 128)]
                        for hh, hsz in spans:
                            if relu_ct % 2 < 1:
                                nc.scalar.activation(
                                    out=hsb[:, hh:hh + hsz],
                                    in_=hps[:, hh:hh + hsz], func=RELU)
                            else:
                                nc.vector.tensor_scalar_max(
                                    hsb[:, hh:hh + hsz],
                                    hps[:, hh:hh + hsz], 0.0)
                            relu_ct += 1
                        pend.append((c0, n, hsb))
                        if len(pend) > MM2_SKEW:
                            emit_mm2(*pend.pop(0))
                        flush(gi)
                        gi += 1
                    while pend:
                        emit_mm2(*pend.pop(0))
                        flush(gi)
                        gi += 1
                while tasks:
                    gi += 1
                    flush(gi)

    nc.compile()
    _NC_CACHE[key] = nc
    return nc


# ------------------------------------------------------------------ driver

def _prepare(inputs):
    x = [np.asarray(inputs["x0"], np.float32),
         np.asarray(inputs["x1"], np.float32)]
    idx = [np.asarray(inputs["idx0"]).astype(np.int64),
           np.asarray(inputs["idx1"]).astype(np.int64)]
    br = [np.asarray(inputs["br0"], np.float32),
          np.asarray(inputs["br1"], np.float32)]
    has_br = bool(np.any(br[0]) or np.any(br[1]))

    plans = [_plan_eye(idx[0]), _plan_eye(idx[1])]
    win_sizes = [plans[0][0], plans[1][0]]
    win_base = [np.cumsum([0] + ws) for ws in win_sizes]
    totals = [int(sum(ws)) for ws in win_sizes]
    n_win = SEGS // WIN

    invc = [
        (1.0 / np.maximum(
            np.bincount(idx[e], minlength=B_FULL), 1)).astype(np.float32)
        for e in range(2)
    ]

    wpk = np.zeros((128, _WCOLS), np.float32)
    for e in range(2):
        wr = np.asarray(inputs[f"Wr{e}"], np.float32)
        wpk[:, _WOFF[f"wr{e}"]:_WOFF[f"wr{e}"] + 128] = \
            np.concatenate([wr, wr], axis=0)
        wpk[:, _WOFF[f"wc{e}"]:_WOFF[f"wc{e}"] + 128] = \
            np.asarray(inputs[f"Wc{e}"], np.float32)
    wb1 = np.asarray(inputs["Wb1"], np.float32)
    wb2 = np.asarray(inputs["Wb2"], np.float32)
    wpk[:, _WOFF["wb1lo"]:_WOFF["wb1lo"] + 256] = wb1[0:128]
    wpk[:, _WOFF["wb1hi"]:_WOFF["wb1hi"] + 256] = wb1[128:256]
    wpk[:, _WOFF["wb2lo"]:_WOFF["wb2lo"] + 128] = wb2[0:128]
    wpk[:, _WOFF["wb2hi"]:_WOFF["wb2hi"] + 128] = wb2[128:256]

    bb1 = np.asarray(inputs["bb1"], np.float32)
    bpk = np.zeros((128, 5), np.float32)
    bpk[:, _BOFF["bc0"]] = np.asarray(inputs["bc0"], np.float32)
    bpk[:, _BOFF["bc1"]] = np.asarray(inputs["bc1"], np.float32)
    bpk[:, _BOFF["bb1a"]] = bb1[0:128]
    bpk[:, _BOFF["bb1b"]] = bb1[128:256]
    bpk[:, _BOFF["bb2"]] = np.asarray(inputs["bb2"], np.float32)

    shared = {"wpk": wpk.astype(BF16), "bpk": bpk}
    if has_br:
        for e in range(2):
            shared[f"br{e}"] = br[e].astype(BF16).reshape(1, -1)

    in_maps = []
    for c in range(CORES):
        m = dict(shared)
        for e in range(2):
            nodes, srel = plans[e][1][c]
            total = totals[e]
            nchunks = total // 128
            wid = srel // WIN
            wstart = np.searchsorted(wid, np.arange(n_win))
            pos = np.empty(len(nodes), np.int64)
            for w in range(n_win):
                lo = wstart[w]
                hi = wstart[w + 1] if w + 1 < n_win else len(nodes)
                pos[lo:hi] = win_base[e][w] + np.arange(hi - lo)
            arr = np.zeros((total, IN_NF), np.float32)
            arr[pos] = x[e][nodes] * invc[e][idx[e][nodes]][:, None]
            a3 = arr.reshape(nchunks, 128, IN_NF).astype(BF16)
            xp = np.concatenate([a3[0::2], a3[1::2]], axis=2)
            m[f"x{e}p"] = np.ascontiguousarray(
                xp.transpose(2, 0, 1).reshape(128, total // 2))
            segv = np.full(total, -1.0, np.float32)
            segv[pos] = (srel % WIN).astype(np.float32)
            sel = (segv[:, None] == np.arange(WIN, dtype=np.float32)[None, :])
            m[f"sel{e}"] = np.ascontiguousarray(
                sel.reshape(nchunks, 128, WIN).transpose(1, 0, 2)
                .reshape(128, nchunks * WIN)).astype(FP8)
            if has_br:
                iv = np.zeros(total, np.float32)
                iv[pos] = invc[e][idx[e][nodes]]
                m[f"invr{e}"] = iv.reshape(1, total)
        in_maps.append(m)
    key = (tuple(win_sizes[0]), tuple(win_sizes[1]), has_br)
    return key, in_maps


def _axon_reset():
    try:
        import ctypes

        lib = ctypes.CDLL("/opt/axon/libaxon_pjrt.so")
        lib.axon_reset.restype = ctypes.c_int
        lib.axon_reset()
    except Exception:
        pass


def _run(inputs, trace=False, trace_kwargs=None):
    key, in_maps = _prepare(inputs)
    nc = _build_nc(key)
    try:
        res = run_bass_kernel_spmd(nc, in_maps, list(range(CORES)),
                                   trace=trace, **(trace_kwargs or {}))
    except Exception as e:
        if "UNRECOVERABLE" not in str(e) and "UNAVAILABLE" not in str(e):
            raise
        _axon_reset()
        res = run_bass_kernel_spmd(nc, in_maps, list(range(CORES)),
                                   trace=trace, **(trace_kwargs or {}))
    out = np.concatenate([res.results[c]["outT"].T for c in range(CORES)],
                         axis=0)
    return out.astype(np.float32), res


def kernel(**inputs):
    return _run(inputs)[0]


# revision 17
# speedup vs baseline: 1.5917x; 1.0445x over previous
"""Trainium2 Bass kernel for nn_NeuralDevice (segment_reduce), v4.

Architecture (per reference):
  two "eyes": h = relu(x @ Wr + br)            [N=1M, 64] -> [N, 128]
              segment-mean over idx (B=65536)  -> [B, 128]
              e = relu(mean @ Wc + bc)         -> [B, 128]
  brain:      z = [e0, e1]; out = relu(z@Wb1+bb1) @ Wb2 + bb2 -> [B, 128]

Distribution: shuffle-by-key, 8 cores x 8192 segments.  Host sorts each
core's nodes by segment, prescales each row by 1/max(cnt,1) (segment SUM
== segment MEAN on device), packs chunk PAIRS into the 128 partitions
(row-tiled K=64 matmul pairs, outputs split across PSUM banks), and
ships the one-hot row->segment selector as fp8 (exact 0/1).

Per 128-row chunk: mm1 pair-packed -> h psum; relu (ACT/DVE 1:1)
-> h bf16 SBUF; mm2: meanT_psum[128 feats, 64 segs] += h^T @ sel_fp8.
8-window groups (512 segs) finish: DVE copy psum->sbuf bf16 = meanT,
eT = relu(Wc^T meanT + bc).  Brain MLP interleaved into eye-1 group
fins.  All cross-engine consumers are emitted 1-3 batches after their
producers (task queue) so the PE FIFO never waits on ACT/DVE latency.
"""

import numpy as np
import ml_dtypes

from concourse import bass, mybir
import concourse.bacc as bacc
import concourse.tile as tile
from concourse.bass_utils import run_bass_kernel_spmd

BF16 = ml_dtypes.bfloat16
FP8 = ml_dtypes.float8_e4m3fn

B_FULL = 65536
N_FULL = 1048576
IN_NF = 64
R_OUT = 128
C_OUT = 128
BRAIN_H = 256
BRAIN_OUT = 128

CORES = 8
SEGS = B_FULL // CORES      # 8192 segments per core
WIN = 64                    # segments per accumulation window
WGRP = 8                    # windows per PSUM group (512 segs)
HB = 8                      # chunks per h-psum batch / relu batch
XCOLS = 4096                # packed-x columns per DMA tile (16 pairs)
SELCH = 128                 # chunks per sel DMA tile
MM2_SKEW = 3                # batches between mm1 and its mm2 consumption

f32 = mybir.dt.float32
bf16 = mybir.dt.bfloat16
fp8e4 = mybir.dt.float8e4
RELU = mybir.ActivationFunctionType.Relu


# ----------------------------------------------------------------- planning

def _plan_eye(idx):
    """Per-eye shared window schedule + per-core sorted node placement."""
    n_win = SEGS // WIN
    owner = idx // SEGS
    per_c = {}
    runs = np.zeros((CORES, n_win), np.int64)
    for c in range(CORES):
        nodes = np.flatnonzero(owner == c)
        srel = idx[nodes] - c * SEGS
        order = np.argsort(srel, kind="stable")
        nodes = nodes[order]
        srel = srel[order]
        per_c[c] = (nodes, srel)
        runs[c] = np.bincount(srel // WIN, minlength=n_win)
    win_sizes = ((runs.max(axis=0) + 127) // 128) * 128
    win_sizes = np.maximum(win_sizes, 128)
    if (int(win_sizes.sum()) // 128) % 2:
        win_sizes[-1] += 128
    return win_sizes.tolist(), per_c


def _eye_sched(win_sizes):
    woc = []
    for w, sz in enumerate(win_sizes):
        woc.extend([w] * (sz // 128))
    first = {}
    last = {}
    for c, w in enumerate(woc):
        first.setdefault(w, c)
        last[w] = c
    return woc, first, last


# ------------------------------------------------------------ program build

_NC_CACHE = {}

# packed bf16 weight layout: [128, 1280]
_WOFF = {"wr0": 0, "wr1": 128, "wc0": 256, "wc1": 384, "wb1lo": 512,
         "wb1hi": 768, "wb2lo": 1024, "wb2hi": 1152}
_WCOLS = 1280
# packed f32 bias layout: [128, 5]
_BOFF = {"bc0": 0, "bc1": 1, "bb1a": 2, "bb1b": 3, "bb2": 4}


def _build_nc(key):
    if key in _NC_CACHE:
        return _NC_CACHE[key]
    (ws0, ws1, has_br) = key
    win_sizes = [list(ws0), list(ws1)]
    scheds = [_eye_sched(win_sizes[0]), _eye_sched(win_sizes[1])]
    nchunks = [len(scheds[0][0]), len(scheds[1][0])]

    nc = bacc.Bacc("TRN2", target_bir_lowering=False, debug=False)

    xp_d = [nc.dram_tensor(f"x{e}p", [128, nchunks[e] * 64], bf16,
                           kind="ExternalInput") for e in range(2)]
    sel_d = [nc.dram_tensor(f"sel{e}", [128, nchunks[e] * WIN], fp8e4,
                            kind="ExternalInput") for e in range(2)]
    wpk_d = nc.dram_tensor("wpk", [128, _WCOLS], bf16, kind="ExternalInput")
    bpk_d = nc.dram_tensor("bpk", [128, 5], f32, kind="ExternalInput")
    if has_br:
        invr_d = [nc.dram_tensor(f"invr{e}", [1, nchunks[e] * 128], f32,
                                 kind="ExternalInput") for e in range(2)]
        br_d = [nc.dram_tensor(f"br{e}", [1, R_OUT], bf16,
                               kind="ExternalInput") for e in range(2)]
    outT_d = nc.dram_tensor("outT", [128, SEGS], f32, kind="ExternalOutput")

    with tile.TileContext(nc) as tc:
        with tc.tile_pool(name="consts", bufs=1) as cp:
            wpk_t = cp.tile([128, _WCOLS], bf16, tag="wpk")
            bpk_t = cp.tile([128, 5], f32, tag="bpk")
            nc.sync.dma_start(out=wpk_t[:], in_=wpk_d[:])
            nc.sync.dma_start(out=bpk_t[:], in_=bpk_d[:])

            def W(name, w=128):
                o = _WOFF[name]
                return wpk_t[:, o:o + w]

            def BIAS(name):
                o = _BOFF[name]
                return bpk_t[:, o:o + 1]

            if has_br:
                br_t = [cp.tile([1, R_OUT], bf16, tag=f"br{e}",
                                name=f"br{e}t") for e in range(2)]
                for e in range(2):
                    nc.sync.dma_start(out=br_t[e][:], in_=br_d[e][:])

            eT_t = [cp.tile([128, SEGS], bf16, tag=f"eT{e}", name=f"eT{e}t")
                    for e in range(2)]

            with (
                tc.tile_pool(name="xch", bufs=4) as xpool,
                tc.tile_pool(name="selp", bufs=4) as selp,
                tc.tile_pool(name="hs", bufs=6) as hpool,
                tc.tile_pool(name="fins", bufs=2) as fs,
                tc.tile_pool(name="invp", bufs=2) as invp,
                tc.tile_pool(name="bs", bufs=3) as bs,
                tc.tile_pool(name="hps", bufs=2, space="PSUM") as hpp,
                tc.tile_pool(name="winp", bufs=2, space="PSUM") as wpp,
                tc.tile_pool(name="wcp", bufs=1, space="PSUM") as wcp,
                tc.tile_pool(name="bph", bufs=1, space="PSUM") as bph,
            ):
                relu_ct = 0
                gi = 0                    # global batch iteration counter
                tasks = []                # (due_gi, fn) queue

                def flush(now):
                    i = 0
                    while i < len(tasks):
                        due, fn = tasks[i]
                        if due <= now:
                            tasks.pop(i)
                            fn()
                            i = 0
                        else:
                            i += 1

                # --------------- brain (split into 3 pipeline tasks)
                def brain_a(t):
                    def fn():
                        r0 = t * 512
                        psh_a = bph.tile([128, 512], f32, tag="bph",
                                         name=f"pha{t}")
                        nc.tensor.matmul(out=psh_a[:], lhsT=W("wb1lo"),
                                         rhs=eT_t[0][:, r0:r0 + 512],
                                         start=True, stop=False)
                        nc.tensor.matmul(out=psh_a[:], lhsT=W("wb1hi"),
                                         rhs=eT_t[1][:, r0:r0 + 512],
                                         start=False, stop=True)
                        hTa = bs.tile([128, 512], bf16, tag="hTa",
                                      name=f"hTa{t}")
                        nc.scalar.activation(out=hTa[:], in_=psh_a[:],
                                             func=RELU, bias=BIAS("bb1a"))
                        fn.hTa = hTa
                    return fn

                def brain_b(t, fa):
                    def fn():
                        r0 = t * 512
                        psh_b = bph.tile([128, 512], f32, tag="bph",
                                         name=f"phb{t}")
                        nc.tensor.matmul(
                            out=psh_b[:],
                            lhsT=wpk_t[:, _WOFF["wb1lo"] + 128:
                                       _WOFF["wb1lo"] + 256],
                            rhs=eT_t[0][:, r0:r0 + 512],
                            start=True, stop=False)
                        nc.tensor.matmul(
                            out=psh_b[:],
                            lhsT=wpk_t[:, _WOFF["wb1hi"] + 128:
                                       _WOFF["wb1hi"] + 256],
                            rhs=eT_t[1][:, r0:r0 + 512],
                            start=False, stop=True)
                        hTb = bs.tile([128, 512], bf16, tag="hTb",
                                      name=f"hTb{t}")
                        nc.scalar.activation(out=hTb[:], in_=psh_b[:],
                                             func=RELU, bias=BIAS("bb1b"))
                        fn.hTb = hTb
                    return fn

                def brain_c(t, fa, fb):
                    def fn():
                        r0 = t * 512
                        psy = bph.tile([128, 512], f32, tag="bph",
                                       name=f"py{t}")
                        nc.tensor.matmul(out=psy[:], lhsT=W("wb2lo"),
                                         rhs=fa.hTa[:], start=True,
                                         stop=False)
                        nc.tensor.matmul(out=psy[:], lhsT=W("wb2hi"),
                                         rhs=fb.hTb[:], start=False,
                                         stop=True)
                        ys = bs.tile([128, 512], f32, tag="ys",
                                     name=f"ys{t}")
                        nc.vector.tensor_scalar_add(ys[:], psy[:],
                                                    BIAS("bb2"))
                        nc.sync.dma_start(out=outT_d[:, r0:r0 + 512],
                                          in_=ys[:])
                    return fn

                # --------------- eye fin: wc matmul + eT relu
                def fin_wc(e, g, meanT):
                    def fn():
                        pse = wcp.tile([128, WGRP * WIN], f32, tag="pse",
                                       name=f"pse{e}_{g}")
                        nc.tensor.matmul(out=pse[:], lhsT=W(f"wc{e}"),
                                         rhs=meanT[:], start=True, stop=True)
                        nc.scalar.activation(
                            out=eT_t[e][:, g * 512:(g + 1) * 512],
                            in_=pse[:], func=RELU, bias=BIAS(f"bc{e}"))
                        if e == 1:
                            fa = brain_a(g)
                            fb = brain_b(g, fa)
                            fc = brain_c(g, fa, fb)
                            tasks.append((gi + 1, fa))
                            tasks.append((gi + 2, fb))
                            tasks.append((gi + 3, fc))
                    return fn

                for e in range(2):
                    woc, wfirst, wlast = scheds[e]
                    nch = nchunks[e]
                    xt = None
                    invt = None
                    selt = None
                    wacc = None
                    pend = []          # batches awaiting mm2 emission

                    def emit_mm2(c0, n, hsb):
                        nonlocal wacc, selt
                        for j in range(n):
                            c = c0 + j
                            w = woc[c]
                            g = w // WGRP
                            if c % SELCH == 0:
                                scnt = min(SELCH, nch - c)
                                selt = selp.tile([128, SELCH * WIN], fp8e4,
                                                 tag="selt",
                                                 name=f"selt{e}_{c}")
                                nc.sync.dma_start(
                                    out=selt[:, :scnt * WIN],
                                    in_=sel_d[e][:, c * WIN:(c + scnt) * WIN])
                            off = (c % SELCH) * WIN
                            if c == wfirst[g * WGRP]:
                                wacc = wpp.tile([128, WGRP * WIN], f32,
                                                tag="wacc", name=f"wa{e}_{g}")
                            ws = (w % WGRP) * WIN
                            slot = (j >> 1) + (j & 1) * (HB // 2)
                            nc.tensor.matmul(
                                out=wacc[:, ws:ws + WIN],
                                lhsT=hsb[:, slot * 128:(slot + 1) * 128],
                                rhs=selt[:, off:off + WIN],
                                start=(c == wfirst[w]),
                                stop=(c == wlast[w]),
                            )
                            if c == wlast[w] and w % WGRP == WGRP - 1:
                                meanT = fs.tile([128, WGRP * WIN], bf16,
                                                tag="meanT", name=f"mt{e}_{g}")
                                nc.vector.tensor_copy(meanT[:], wacc[:])
                                tasks.append((gi + 1, fin_wc(e, g, meanT)))

                    for c0 in range(0, nch, HB):
                        n = min(HB, nch - c0)
                        hps = hpp.tile([128, HB * 128], f32, tag="hps",
                                       name=f"hps{e}_{c0}")
                        hsb = hpool.tile([128, HB * 128], bf16, tag="hsb",
                                         name=f"hsb{e}_{c0}")
                        for t in range(n // 2):
                            pair = c0 // 2 + t
                            if pair % (XCOLS // 128) == 0:
                                pbase = pair * 128
                                pcsz = min(XCOLS, nch * 64 - pbase)
                                xt = xpool.tile([128, XCOLS], bf16, tag="xch",
                                                name=f"xch{e}_{pair}")
                                nc.sync.dma_start(
                                    out=xt[:, :pcsz],
                                    in_=xp_d[e][:, pbase:pbase + pcsz])
                                if has_br:
                                    ibase = pair * 256
                                    icsz = min(2 * XCOLS,
                                               nch * 128 - ibase)
                                    invt = invp.tile([1, 2 * XCOLS], f32,
                                                     tag="invr",
                                                     name=f"invr{e}_{pair}")
                                    nc.sync.dma_start(
                                        out=invt[:, :icsz],
                                        in_=invr_d[e][:, ibase:ibase + icsz])
                            col = (pair % (XCOLS // 128)) * 128
                            for half in range(2):
                                # row-tiled pair: A -> bank0 slot t,
                                # B -> bank1 slot HB//2+t (concurrent row
                                # tiles must write different PSUM banks)
                                slot = t + half * (HB // 2)
                                hs = slice(slot * 128, (slot + 1) * 128)
                                pb = half * 64
                                nc.tensor.matmul(
                                    out=hps[:, hs],
                                    lhsT=xt[pb:pb + 64, col:col + 128],
                                    rhs=W(f"wr{e}")[pb:pb + 64, :],
                                    start=True, stop=not has_br,
                                )
                                if has_br:
                                    ic = (pair % (XCOLS // 128)) * 256 \
                                        + half * 128
                                    nc.tensor.matmul(
                                        out=hps[:, hs],
                                        lhsT=invt[0:1, ic:ic + 128],
                                        rhs=br_t[e][0:1, :],
                                        start=False, stop=True,
                                    )
                        if n == HB:
                            spans = [(0, HB * 128)]
                        else:
                            spans = [(0, (n // 2) * 128),
                                     ((HB // 2) * 128, (n // 2) * 128)]
                        for hh, hsz in spans:
                            if relu_ct % 2 < 1:
                                nc.scalar.activation(
                                    out=hsb[:, hh:hh + hsz],
                                    in_=hps[:, hh:hh + hsz], func=RELU)
                            else:
                                nc.vector.tensor_scalar_max(
                                    hsb[:, hh:hh + hsz],
                                    hps[:, hh:hh + hsz], 0.0)
                            relu_ct += 1
                        pend.append((c0, n, hsb))
                        flush(gi)
                        if len(pend) > MM2_SKEW:
                            emit_mm2(*pend.pop(0))
                        gi += 1
                    while pend:
                        emit_mm2(*pend.pop(0))
                        flush(gi)
                        gi += 1
                while tasks:
                    gi += 1
                    flush(gi)

    nc.compile()
    _NC_CACHE[key] = nc
    return nc


# ------------------------------------------------------------------ driver

def _prepare(inputs):
    x = [np.asarray(inputs["x0"], np.float32),
         np.asarray(inputs["x1"], np.float32)]
    idx = [np.asarray(inputs["idx0"]).astype(np.int64),
           np.asarray(inputs["idx1"]).astype(np.int64)]
    br = [np.asarray(inputs["br0"], np.float32),
          np.asarray(inputs["br1"], np.float32)]
    has_br = bool(np.any(br[0]) or np.any(br[1]))

    plans = [_plan_eye(idx[0]), _plan_eye(idx[1])]
    win_sizes = [plans[0][0], plans[1][0]]
    win_base = [np.cumsum([0] + ws) for ws in win_sizes]
    totals = [int(sum(ws)) for ws in win_sizes]
    n_win = SEGS // WIN

    invc = [
        (1.0 / np.maximum(
            np.bincount(idx[e], minlength=B_FULL), 1)).astype(np.float32)
        for e in range(2)
    ]

    wpk = np.zeros((128, _WCOLS), np.float32)
    for e in range(2):
        wr = np.asarray(inputs[f"Wr{e}"], np.float32)
        wpk[:, _WOFF[f"wr{e}"]:_WOFF[f"wr{e}"] + 128] = \
            np.concatenate([wr, wr], axis=0)
        wpk[:, _WOFF[f"wc{e}"]:_WOFF[f"wc{e}"] + 128] = \
            np.asarray(inputs[f"Wc{e}"], np.float32)
    wb1 = np.asarray(inputs["Wb1"], np.float32)
    wb2 = np.asarray(inputs["Wb2"], np.float32)
    wpk[:, _WOFF["wb1lo"]:_WOFF["wb1lo"] + 256] = wb1[0:128]
    wpk[:, _WOFF["wb1hi"]:_WOFF["wb1hi"] + 256] = wb1[128:256]
    wpk[:, _WOFF["wb2lo"]:_WOFF["wb2lo"] + 128] = wb2[0:128]
    wpk[:, _WOFF["wb2hi"]:_WOFF["wb2hi"] + 128] = wb2[128:256]

    bb1 = np.asarray(inputs["bb1"], np.float32)
    bpk = np.zeros((128, 5), np.float32)
    bpk[:, _BOFF["bc0"]] = np.asarray(inputs["bc0"], np.float32)
    bpk[:, _BOFF["bc1"]] = np.asarray(inputs["bc1"], np.float32)
    bpk[:, _BOFF["bb1a"]] = bb1[0:128]
    bpk[:, _BOFF["bb1b"]] = bb1[128:256]
    bpk[:, _BOFF["bb2"]] = np.asarray(inputs["bb2"], np.float32)

    shared = {"wpk": wpk.astype(BF16), "bpk": bpk}
    if has_br:
        for e in range(2):
            shared[f"br{e}"] = br[e].astype(BF16).reshape(1, -1)

    in_maps = []
    for c in range(CORES):
        m = dict(shared)
        for e in range(2):
            nodes, srel = plans[e][1][c]
            total = totals[e]
            nchunks = total // 128
            wid = srel // WIN
            wstart = np.searchsorted(wid, np.arange(n_win))
            pos = np.empty(len(nodes), np.int64)
            for w in range(n_win):
                lo = wstart[w]
                hi = wstart[w + 1] if w + 1 < n_win else len(nodes)
                pos[lo:hi] = win_base[e][w] + np.arange(hi - lo)
            arr = np.zeros((total, IN_NF), np.float32)
            arr[pos] = x[e][nodes] * invc[e][idx[e][nodes]][:, None]
            a3 = arr.reshape(nchunks, 128, IN_NF).astype(BF16)
            xp = np.concatenate([a3[0::2], a3[1::2]], axis=2)
            m[f"x{e}p"] = np.ascontiguousarray(
                xp.transpose(2, 0, 1).reshape(128, total // 2))
            segv = np.full(total, -1.0, np.float32)
            segv[pos] = (srel % WIN).astype(np.float32)
            sel = (segv[:, None] == np.arange(WIN, dtype=np.float32)[None, :])
            m[f"sel{e}"] = np.ascontiguousarray(
                sel.reshape(nchunks, 128, WIN).transpose(1, 0, 2)
                .reshape(128, nchunks * WIN)).astype(FP8)
            if has_br:
                iv = np.zeros(total, np.float32)
                iv[pos] = invc[e][idx[e][nodes]]
                m[f"invr{e}"] = iv.reshape(1, total)
        in_maps.append(m)
    key = (tuple(win_sizes[0]), tuple(win_sizes[1]), has_br)
    return key, in_maps


def _axon_reset():
    try:
        import ctypes

        lib = ctypes.CDLL("/opt/axon/libaxon_pjrt.so")
        lib.axon_reset.restype = ctypes.c_int
        lib.axon_reset()
    except Exception:
        pass


def _run(inputs, trace=False, trace_kwargs=None):
    key, in_maps = _prepare(inputs)
    nc = _build_nc(key)
    try:
        res = run_bass_kernel_spmd(nc, in_maps, list(range(CORES)),
                                   trace=trace, **(trace_kwargs or {}))
    except Exception as e:
        if "UNRECOVERABLE" not in str(e) and "UNAVAILABLE" not in str(e):
            raise
        _axon_reset()
        res = run_bass_kernel_spmd(nc, in_maps, list(range(CORES)),
                                   trace=trace, **(trace_kwargs or {}))
    out = np.concatenate([res.results[c]["outT"].T for c in range(CORES)],
                         axis=0)
    return out.astype(np.float32), res


def kernel(**inputs):
    return _run(inputs)[0]


# revision 22
# speedup vs baseline: 1.6681x; 1.0480x over previous
"""Trainium2 Bass kernel for nn_NeuralDevice (segment_reduce), v4.

Architecture (per reference):
  two "eyes": h = relu(x @ Wr + br)            [N=1M, 64] -> [N, 128]
              segment-mean over idx (B=65536)  -> [B, 128]
              e = relu(mean @ Wc + bc)         -> [B, 128]
  brain:      z = [e0, e1]; out = relu(z@Wb1+bb1) @ Wb2 + bb2 -> [B, 128]

Distribution: shuffle-by-key, 8 cores x 8192 segments.  Host sorts each
core's nodes by segment, prescales each row by 1/max(cnt,1) (segment SUM
== segment MEAN on device), packs chunk PAIRS into the 128 partitions
(row-tiled K=64 matmul pairs, outputs split across PSUM banks), and
ships the one-hot row->segment selector as fp8 (exact 0/1).

Per 128-row chunk: mm1 pair-packed -> h psum; relu (ACT/DVE 1:1)
-> h bf16 SBUF; mm2: meanT_psum[128 feats, 64 segs] += h^T @ sel_fp8.
8-window groups (512 segs) finish: DVE copy psum->sbuf bf16 = meanT,
eT = relu(Wc^T meanT + bc).  Brain MLP interleaved into eye-1 group
fins.  All cross-engine consumers are emitted 1-3 batches after their
producers (task queue) so the PE FIFO never waits on ACT/DVE latency.
"""

import numpy as np
import ml_dtypes

from concourse import bass, mybir
import concourse.bacc as bacc
import concourse.tile as tile
from concourse.bass_utils import run_bass_kernel_spmd

BF16 = ml_dtypes.bfloat16
FP8 = ml_dtypes.float8_e4m3fn

B_FULL = 65536
N_FULL = 1048576
IN_NF = 64
R_OUT = 128
C_OUT = 128
BRAIN_H = 256
BRAIN_OUT = 128

CORES = 8
SEGS = B_FULL // CORES      # 8192 segments per core
WIN = 64                    # segments per accumulation window
WGRP = 8                    # windows per PSUM group (512 segs)
HB = 8                      # chunks per h-psum batch / relu batch
XCOLS = 4096                # packed-x columns per DMA tile (16 pairs)
SELCH = 128                 # chunks per sel DMA tile
MM2_SKEW = 3                # batches between mm1 and its mm2 consumption

f32 = mybir.dt.float32
bf16 = mybir.dt.bfloat16
fp8e4 = mybir.dt.float8e4
RELU = mybir.ActivationFunctionType.Relu


# ----------------------------------------------------------------- planning

def _plan_eye(idx):
    """Per-eye shared window schedule + per-core sorted node placement."""
    n_win = SEGS // WIN
    owner = idx // SEGS
    per_c = {}
    runs = np.zeros((CORES, n_win), np.int64)
    for c in range(CORES):
        nodes = np.flatnonzero(owner == c)
        srel = idx[nodes] - c * SEGS
        order = np.argsort(srel, kind="stable")
        nodes = nodes[order]
        srel = srel[order]
        per_c[c] = (nodes, srel)
        runs[c] = np.bincount(srel // WIN, minlength=n_win)
    win_sizes = np.maximum(runs.max(axis=0), 1)
    return win_sizes.tolist(), per_c


def _eye_sched(win_sizes):
    """Exact-size windows; chunks may straddle two windows (2 mm2 jobs).

    Returns (nchunks, jobs_of_chunk, wfirst, wlast) where
    jobs_of_chunk[c] = list of window ids with rows in chunk c, and
    wfirst/wlast map window -> first/last chunk containing its rows.
    """
    base = np.cumsum([0] + list(win_sizes))
    total = int(-(-base[-1] // 256) * 256)
    nchunks = total // 128
    jobs_of_chunk = [[] for _ in range(nchunks)]
    wfirst = {}
    wlast = {}
    for w, sz in enumerate(win_sizes):
        c0 = int(base[w]) // 128
        c1 = int(base[w] + sz - 1) // 128
        wfirst[w] = c0
        wlast[w] = c1
        for c in range(c0, c1 + 1):
            jobs_of_chunk[c].append(w)
    return nchunks, jobs_of_chunk, wfirst, wlast


# ------------------------------------------------------------ program build

_NC_CACHE = {}

# packed bf16 weight layout: [128, 1280]
_WOFF = {"wr0": 0, "wr1": 128, "wc0": 256, "wc1": 384, "wb1lo": 512,
         "wb1hi": 768, "wb2lo": 1024, "wb2hi": 1152}
_WCOLS = 1280
# packed f32 bias layout: [128, 5]
_BOFF = {"bc0": 0, "bc1": 1, "bb1a": 2, "bb1b": 3, "bb2": 4}


def _build_nc(key):
    if key in _NC_CACHE:
        return _NC_CACHE[key]
    (ws0, ws1, has_br) = key
    win_sizes = [list(ws0), list(ws1)]
    scheds = [_eye_sched(win_sizes[0]), _eye_sched(win_sizes[1])]
    nchunks = [scheds[0][0], scheds[1][0]]
    njobs = [sum(len(js) for js in scheds[e][1]) for e in range(2)]

    nc = bacc.Bacc("TRN2", target_bir_lowering=False, debug=False)

    xp_d = [nc.dram_tensor(f"x{e}p", [128, nchunks[e] * 64], bf16,
                           kind="ExternalInput") for e in range(2)]
    sel_d = [nc.dram_tensor(f"sel{e}", [128, njobs[e] * WIN], fp8e4,
                            kind="ExternalInput") for e in range(2)]
    wpk_d = nc.dram_tensor("wpk", [128, _WCOLS], bf16, kind="ExternalInput")
    bpk_d = nc.dram_tensor("bpk", [128, 5], f32, kind="ExternalInput")
    if has_br:
        invr_d = [nc.dram_tensor(f"invr{e}", [1, nchunks[e] * 128], f32,
                                 kind="ExternalInput") for e in range(2)]
        br_d = [nc.dram_tensor(f"br{e}", [1, R_OUT], bf16,
                               kind="ExternalInput") for e in range(2)]
    outT_d = nc.dram_tensor("outT", [128, SEGS], f32, kind="ExternalOutput")

    with tile.TileContext(nc) as tc:
        with tc.tile_pool(name="consts", bufs=1) as cp:
            wpk_t = cp.tile([128, _WCOLS], bf16, tag="wpk")
            bpk_t = cp.tile([128, 5], f32, tag="bpk")
            nc.sync.dma_start(out=wpk_t[:], in_=wpk_d[:])
            nc.sync.dma_start(out=bpk_t[:], in_=bpk_d[:])

            def W(name, w=128):
                o = _WOFF[name]
                return wpk_t[:, o:o + w]

            def BIAS(name):
                o = _BOFF[name]
                return bpk_t[:, o:o + 1]

            if has_br:
                br_t = [cp.tile([1, R_OUT], bf16, tag=f"br{e}",
                                name=f"br{e}t") for e in range(2)]
                for e in range(2):
                    nc.sync.dma_start(out=br_t[e][:], in_=br_d[e][:])

            eT_t = [cp.tile([128, SEGS], bf16, tag=f"eT{e}", name=f"eT{e}t")
                    for e in range(2)]

            with (
                tc.tile_pool(name="xch", bufs=4) as xpool,
                tc.tile_pool(name="selp", bufs=4) as selp,
                tc.tile_pool(name="hs", bufs=6) as hpool,
                tc.tile_pool(name="fins", bufs=2) as fs,
                tc.tile_pool(name="invp", bufs=2) as invp,
                tc.tile_pool(name="bs", bufs=3) as bs,
                tc.tile_pool(name="hps", bufs=2, space="PSUM") as hpp,
                tc.tile_pool(name="winp", bufs=2, space="PSUM") as wpp,
                tc.tile_pool(name="wcp", bufs=1, space="PSUM") as wcp,
                tc.tile_pool(name="bph", bufs=1, space="PSUM") as bph,
            ):
                relu_ct = 0
                gi = 0                    # global batch iteration counter
                tasks = []                # (due_gi, fn) queue

                def flush(now):
                    i = 0
                    while i < len(tasks):
                        due, fn = tasks[i]
                        if due <= now:
                            tasks.pop(i)
                            fn()
                            i = 0
                        else:
                            i += 1

                # --------------- brain (split into 3 pipeline tasks)
                def brain_a(t):
                    def fn():
                        r0 = t * 512
                        psh_a = bph.tile([128, 512], f32, tag="bph",
                                         name=f"pha{t}")
                        nc.tensor.matmul(out=psh_a[:], lhsT=W("wb1lo"),
                                         rhs=eT_t[0][:, r0:r0 + 512],
                                         start=True, stop=False)
                        nc.tensor.matmul(out=psh_a[:], lhsT=W("wb1hi"),
                                         rhs=eT_t[1][:, r0:r0 + 512],
                                         start=False, stop=True)
                        hTa = bs.tile([128, 512], bf16, tag="hTa",
                                      name=f"hTa{t}")
                        nc.scalar.activation(out=hTa[:], in_=psh_a[:],
                                             func=RELU, bias=BIAS("bb1a"))
                        fn.hTa = hTa
                    return fn

                def brain_b(t, fa):
                    def fn():
                        r0 = t * 512
                        psh_b = bph.tile([128, 512], f32, tag="bph",
                                         name=f"phb{t}")
                        nc.tensor.matmul(
                            out=psh_b[:],
                            lhsT=wpk_t[:, _WOFF["wb1lo"] + 128:
                                       _WOFF["wb1lo"] + 256],
                            rhs=eT_t[0][:, r0:r0 + 512],
                            start=True, stop=False)
                        nc.tensor.matmul(
                            out=psh_b[:],
                            lhsT=wpk_t[:, _WOFF["wb1hi"] + 128:
                                       _WOFF["wb1hi"] + 256],
                            rhs=eT_t[1][:, r0:r0 + 512],
                            start=False, stop=True)
                        hTb = bs.tile([128, 512], bf16, tag="hTb",
                                      name=f"hTb{t}")
                        nc.scalar.activation(out=hTb[:], in_=psh_b[:],
                                             func=RELU, bias=BIAS("bb1b"))
                        fn.hTb = hTb
                    return fn

                def brain_c(t, fa, fb):
                    def fn():
                        r0 = t * 512
                        psy = bph.tile([128, 512], f32, tag="bph",
                                       name=f"py{t}")
                        nc.tensor.matmul(out=psy[:], lhsT=W("wb2lo"),
                                         rhs=fa.hTa[:], start=True,
                                         stop=False)
                        nc.tensor.matmul(out=psy[:], lhsT=W("wb2hi"),
                                         rhs=fb.hTb[:], start=False,
                                         stop=True)
                        ys = bs.tile([128, 512], f32, tag="ys",
                                     name=f"ys{t}")
                        nc.vector.tensor_scalar_add(ys[:], psy[:],
                                                    BIAS("bb2"))
                        nc.sync.dma_start(out=outT_d[:, r0:r0 + 512],
                                          in_=ys[:])
                    return fn

                # --------------- eye fin: wc matmul + eT relu
                def fin_wc(e, g, meanT):
                    def fn():
                        pse = wcp.tile([128, WGRP * WIN], f32, tag="pse",
                                       name=f"pse{e}_{g}")
                        nc.tensor.matmul(out=pse[:], lhsT=W(f"wc{e}"),
                                         rhs=meanT[:], start=True, stop=True)
                        nc.scalar.activation(
                            out=eT_t[e][:, g * 512:(g + 1) * 512],
                            in_=pse[:], func=RELU, bias=BIAS(f"bc{e}"))
                        if e == 1:
                            fa = brain_a(g)
                            fb = brain_b(g, fa)
                            fc = brain_c(g, fa, fb)
                            tasks.append((gi + 1, fa))
                            tasks.append((gi + 2, fb))
                            tasks.append((gi + 3, fc))
                    return fn

                for e in range(2):
                    nch, jobs_of_chunk, wfirst, wlast = scheds[e]
                    njob = njobs[e]
                    xt = None
                    invt = None
                    selt = None
                    wacc = None
                    jid = 0            # running job index (sel layout order)
                    pend = []          # batches awaiting mm2 emission

                    def emit_mm2(c0, n, hsb):
                        nonlocal wacc, selt, jid
                        for j in range(n):
                            c = c0 + j
                            slot = (j >> 1) + (j & 1) * (HB // 2)
                            for w in jobs_of_chunk[c]:
                                g = w // WGRP
                                if jid % SELCH == 0:
                                    scnt = min(SELCH, njob - jid)
                                    selt = selp.tile([128, SELCH * WIN],
                                                     fp8e4, tag="selt",
                                                     name=f"selt{e}_{jid}")
                                    nc.sync.dma_start(
                                        out=selt[:, :scnt * WIN],
                                        in_=sel_d[e][:, jid * WIN:
                                                     (jid + scnt) * WIN])
                                off = (jid % SELCH) * WIN
                                jid += 1
                                if w == g * WGRP and c == wfirst[w]:
                                    wacc = wpp.tile([128, WGRP * WIN], f32,
                                                    tag="wacc",
                                                    name=f"wa{e}_{g}")
                                ws = (w % WGRP) * WIN
                                nc.tensor.matmul(
                                    out=wacc[:, ws:ws + WIN],
                                    lhsT=hsb[:, slot * 128:(slot + 1) * 128],
                                    rhs=selt[:, off:off + WIN],
                                    start=(c == wfirst[w]),
                                    stop=(c == wlast[w]),
                                )
                                if c == wlast[w] and w % WGRP == WGRP - 1:
                                    meanT = fs.tile([128, WGRP * WIN], bf16,
                                                    tag="meanT",
                                                    name=f"mt{e}_{g}")
                                    nc.vector.tensor_copy(meanT[:], wacc[:])
                                    tasks.append((gi + 1,
                                                  fin_wc(e, g, meanT)))

                    for c0 in range(0, nch, HB):
                        n = min(HB, nch - c0)
                        hps = hpp.tile([128, HB * 128], f32, tag="hps",
                                       name=f"hps{e}_{c0}")
                        hsb = hpool.tile([128, HB * 128], bf16, tag="hsb",
                                         name=f"hsb{e}_{c0}")
                        for t in range(n // 2):
                            pair = c0 // 2 + t
                            if pair % (XCOLS // 128) == 0:
                                pbase = pair * 128
                                pcsz = min(XCOLS, nch * 64 - pbase)
                                xt = xpool.tile([128, XCOLS], bf16, tag="xch",
                                                name=f"xch{e}_{pair}")
                                nc.sync.dma_start(
                                    out=xt[:, :pcsz],
                                    in_=xp_d[e][:, pbase:pbase + pcsz])
                                if has_br:
                                    ibase = pair * 256
                                    icsz = min(2 * XCOLS,
                                               nch * 128 - ibase)
                                    invt = invp.tile([1, 2 * XCOLS], f32,
                                                     tag="invr",
                                                     name=f"invr{e}_{pair}")
                                    nc.sync.dma_start(
                                        out=invt[:, :icsz],
                                        in_=invr_d[e][:, ibase:ibase + icsz])
                            col = (pair % (XCOLS // 128)) * 128
                            for half in range(2):
                                # row-tiled pair: A -> bank0 slot t,
                                # B -> bank1 slot HB//2+t (concurrent row
                                # tiles must write different PSUM banks)
                                slot = t + half * (HB // 2)
                                hs = slice(slot * 128, (slot + 1) * 128)
                                pb = half * 64
                                nc.tensor.matmul(
                                    out=hps[:, hs],
                                    lhsT=xt[pb:pb + 64, col:col + 128],
                                    rhs=W(f"wr{e}")[pb:pb + 64, :],
                                    start=True, stop=not has_br,
                                )
                                if has_br:
                                    ic = (pair % (XCOLS // 128)) * 256 \
                                        + half * 128
                                    nc.tensor.matmul(
                                        out=hps[:, hs],
                                        lhsT=invt[0:1, ic:ic + 128],
                                        rhs=br_t[e][0:1, :],
                                        start=False, stop=True,
                                    )
                        if n == HB:
                            spans = [(0, HB * 128)]
                        else:
                            spans = [(0, (n // 2) * 128),
                                     ((HB // 2) * 128, (n // 2) * 128)]
                        for hh, hsz in spans:
                            if relu_ct % 2 < 1:
                                nc.scalar.activation(
                                    out=hsb[:, hh:hh + hsz],
                                    in_=hps[:, hh:hh + hsz], func=RELU)
                            else:
                                nc.vector.tensor_scalar_max(
                                    hsb[:, hh:hh + hsz],
                                    hps[:, hh:hh + hsz], 0.0)
                            relu_ct += 1
                        pend.append((c0, n, hsb))
                        flush(gi)
                        if len(pend) > MM2_SKEW:
                            emit_mm2(*pend.pop(0))
                        gi += 1
                    while pend:
                        emit_mm2(*pend.pop(0))
                        flush(gi)
                        gi += 1
                while tasks:
                    gi += 1
                    flush(gi)

    nc.compile()
    _NC_CACHE[key] = nc
    return nc


# ------------------------------------------------------------------ driver

def _prepare(inputs):
    x = [np.asarray(inputs["x0"], np.float32),
         np.asarray(inputs["x1"], np.float32)]
    idx = [np.asarray(inputs["idx0"]).astype(np.int64),
           np.asarray(inputs["idx1"]).astype(np.int64)]
    br = [np.asarray(inputs["br0"], np.float32),
          np.asarray(inputs["br1"], np.float32)]
    has_br = bool(np.any(br[0]) or np.any(br[1]))

    plans = [_plan_eye(idx[0]), _plan_eye(idx[1])]
    win_sizes = [plans[0][0], plans[1][0]]
    win_base = [np.cumsum([0] + ws) for ws in win_sizes]
    totals = [int(-(-int(win_base[e][-1]) // 256) * 256) for e in range(2)]
    n_win = SEGS // WIN
    # shared job order: (chunk asc, window asc)
    jobs = []
    for e in range(2):
        nchunks_e, jobs_of_chunk, _, _ = _eye_sched(win_sizes[e])
        jw = []
        jc = []
        for c in range(nchunks_e):
            for w in jobs_of_chunk[c]:
                jc.append(c)
                jw.append(w)
        jobs.append((np.array(jc), np.array(jw)))

    invc = [
        (1.0 / np.maximum(
            np.bincount(idx[e], minlength=B_FULL), 1)).astype(np.float32)
        for e in range(2)
    ]

    wpk = np.zeros((128, _WCOLS), np.float32)
    for e in range(2):
        wr = np.asarray(inputs[f"Wr{e}"], np.float32)
        wpk[:, _WOFF[f"wr{e}"]:_WOFF[f"wr{e}"] + 128] = \
            np.concatenate([wr, wr], axis=0)
        wpk[:, _WOFF[f"wc{e}"]:_WOFF[f"wc{e}"] + 128] = \
            np.asarray(inputs[f"Wc{e}"], np.float32)
    wb1 = np.asarray(inputs["Wb1"], np.float32)
    wb2 = np.asarray(inputs["Wb2"], np.float32)
    wpk[:, _WOFF["wb1lo"]:_WOFF["wb1lo"] + 256] = wb1[0:128]
    wpk[:, _WOFF["wb1hi"]:_WOFF["wb1hi"] + 256] = wb1[128:256]
    wpk[:, _WOFF["wb2lo"]:_WOFF["wb2lo"] + 128] = wb2[0:128]
    wpk[:, _WOFF["wb2hi"]:_WOFF["wb2hi"] + 128] = wb2[128:256]

    bb1 = np.asarray(inputs["bb1"], np.float32)
    bpk = np.zeros((128, 5), np.float32)
    bpk[:, _BOFF["bc0"]] = np.asarray(inputs["bc0"], np.float32)
    bpk[:, _BOFF["bc1"]] = np.asarray(inputs["bc1"], np.float32)
    bpk[:, _BOFF["bb1a"]] = bb1[0:128]
    bpk[:, _BOFF["bb1b"]] = bb1[128:256]
    bpk[:, _BOFF["bb2"]] = np.asarray(inputs["bb2"], np.float32)

    shared = {"wpk": wpk.astype(BF16), "bpk": bpk}
    if has_br:
        for e in range(2):
            shared[f"br{e}"] = br[e].astype(BF16).reshape(1, -1)

    in_maps = []
    for c in range(CORES):
        m = dict(shared)
        for e in range(2):
            nodes, srel = plans[e][1][c]
            total = totals[e]
            nchunks = total // 128
            wid = srel // WIN
            wstart = np.searchsorted(wid, np.arange(n_win))
            pos = np.empty(len(nodes), np.int64)
            for w in range(n_win):
                lo = wstart[w]
                hi = wstart[w + 1] if w + 1 < n_win else len(nodes)
                pos[lo:hi] = win_base[e][w] + np.arange(hi - lo)
            arr = np.zeros((total, IN_NF), np.float32)
            arr[pos] = x[e][nodes] * invc[e][idx[e][nodes]][:, None]
            a3 = arr.reshape(nchunks, 128, IN_NF).astype(BF16)
            xp = np.concatenate([a3[0::2], a3[1::2]], axis=2)
            m[f"x{e}p"] = np.ascontiguousarray(
                xp.transpose(2, 0, 1).reshape(128, total // 2))
            segv = np.full(total, -10 * SEGS, np.int64)
            segv[pos] = srel
            jc, jw = jobs[e]
            # job block j: one-hot of (seg - 64*w_j) over chunk c_j's rows
            rel = segv.reshape(nchunks, 128)[jc] - (jw * WIN)[:, None]
            sel = (rel[:, :, None] ==
                   np.arange(WIN, dtype=np.int64)[None, None, :])
            m[f"sel{e}"] = np.ascontiguousarray(
                sel.transpose(1, 0, 2).reshape(128, len(jc) * WIN)
            ).astype(FP8)
            if has_br:
                iv = np.zeros(total, np.float32)
                iv[pos] = invc[e][idx[e][nodes]]
                m[f"invr{e}"] = iv.reshape(1, total)
        in_maps.append(m)
    key = (tuple(win_sizes[0]), tuple(win_sizes[1]), has_br)
    return key, in_maps


def _axon_reset():
    try:
        import ctypes

        lib = ctypes.CDLL("/opt/axon/libaxon_pjrt.so")
        lib.axon_reset.restype = ctypes.c_int
        lib.axon_reset()
    except Exception:
        pass


def _run(inputs, trace=False, trace_kwargs=None):
    key, in_maps = _prepare(inputs)
    nc = _build_nc(key)
    try:
        res = run_bass_kernel_spmd(nc, in_maps, list(range(CORES)),
                                   trace=trace, **(trace_kwargs or {}))
    except Exception as e:
        if "UNRECOVERABLE" not in str(e) and "UNAVAILABLE" not in str(e):
            raise
        _axon_reset()
        res = run_bass_kernel_spmd(nc, in_maps, list(range(CORES)),
                                   trace=trace, **(trace_kwargs or {}))
    out = np.concatenate([res.results[c]["outT"].T for c in range(CORES)],
                         axis=0)
    return out.astype(np.float32), res


def kernel(**inputs):
    return _run(inputs)[0]
